# revision 1
# baseline (speedup 1.0000x reference)
"""Trainium2 Bass kernel for nn_CombinedLoss (Poisson + 3-way pairwise CLIP loss).

Strategy (8 NeuronCores, SPMD, no collectives):
  - Row-shard the batch: core c owns rows [c*512, (c+1)*512) of every tensor.
  - For each feature pair (a,b) in {(1,2),(1,3),(2,3)} each core computes its
    512x4096 block of S_ab = Za @ Zb^T with bf16 matmuls (fp32 PSUM accum):
      lhsT = raw-cast bf16 own-slice of a, transposed on-chip (PE transpose);
      rhs  = normalized bf16 full feature b, transposed via DMA xbar
             (bf16 roundtrip through a DRAM scratch buffer).
    The 1/||a|| normalization of the lhsT side is folded into the exp's
    per-partition scale on the Scalar engine: exp(S_raw * (2/||a_m||)).
  - Row-wise sum(exp) comes free via the activation's accum_out.
  - Column-wise sum(exp) via ones-vector matmuls (contraction over partitions),
    accumulated in PSUM across the 4 M-tiles; partial per-core, host combines.
  - 1/sqrt on device via bit-trick + 2 Newton steps on the Vector engine
    (avoids ACT Ln/Exp table thrashing; ACT does only Exp + poisson Ln).
  - Diagonal similarities via fused multiply+reduce on own slices (raw dots,
    normalized on host with the device-computed squared norms).
  - Host does only the O(B) final combine: log of 4096-length sums, means.
"""

import os
import sys

import numpy as np

sys.path.insert(0, "/opt/trn_rl_repo")

P = 128
TEMPERATURE = 0.5
EPS_POISSON = 1e-8
RSQRT_MAGIC = 0x5F3759DF


class Cfg:
    def __init__(self, B=4096, D=1024, n_cores=8, ntc=512):
        self.B = B          # batch
        self.D = D          # feature dim
        self.n_cores = n_cores
        self.S = B // n_cores      # own rows per core
        self.MT = self.S // P      # M tiles (own rows / 128)
        self.K = D // P            # contraction tiles
        self.NTC = ntc             # columns per rhs tile
        self.NT = B // ntc         # number of rhs tiles
        self.ST = ntc // P         # row-subtiles per rhs tile
        assert B % n_cores == 0 and self.S % P == 0 and D % P == 0 and B % ntc == 0


def _patch_act_tables():
    """Make Bacc's act-table pass pick `natural_log_exp_and_others` for both
    Exp and Ln (they otherwise land in two different sets, and alternating
    Ln/Exp calls reload the 2.7us activation tables every tile).

    Keeps list order (index == act_func_set_id) but empties the earlier
    exp-only / ln-only sets so the first set containing Exp or Ln is the
    combined one."""
    import functools

    import concourse.hw_specs as hw_specs

    if getattr(hw_specs, "_act_tables_patched", False):
        return
    orig = hw_specs.get_activation_tables

    @functools.cache
    def patched(module_arch):
        tabs = dict(orig(module_arch))
        names = list(tabs.keys())
        if "natural_log_exp_and_others" in tabs:
            combined = tabs["natural_log_exp_and_others"]
            for name in names:
                if name == "natural_log_exp_and_others":
                    break
                if tabs[name] & combined:
                    tabs[name] = tabs[name] - combined
        return tabs

    hw_specs.get_activation_tables = patched
    # bacc imports the symbol lazily via module attr? patch its ref if bound
    import concourse.bacc as bacc_mod

    if hasattr(bacc_mod, "get_activation_tables"):
        bacc_mod.get_activation_tables = patched
    hw_specs._act_tables_patched = True


def build_bass(cfg: Cfg):
    """Build the single-core Bass program (same program for all SPMD cores)."""
    import concourse.bacc as bacc
    import concourse.bass as bass
    import concourse.mybir as mybir
    import concourse.tile as tile
    from concourse.masks import make_identity

    _patch_act_tables()

    f32 = mybir.dt.float32
    bf16 = mybir.dt.bfloat16
    i32 = mybir.dt.int32
    AF = mybir.ActivationFunctionType
    ALU = mybir.AluOpType
    ts = bass.ts

    B, D, K, MT, NT, NTC, ST = cfg.B, cfg.D, cfg.K, cfg.MT, cfg.NT, cfg.NTC, cfg.ST

    nc = bacc.Bacc(
        "TRN2",
        target_bir_lowering=False,
        debug=False,
        enable_asserts=False,
        num_devices=cfg.n_cores,
    )

    # ---- IO ----
    f1o = nc.dram_tensor("f1_own", [cfg.S, D], f32, kind="ExternalInput").ap()
    f2o = nc.dram_tensor("f2_own", [cfg.S, D], f32, kind="ExternalInput").ap()
    f3o = nc.dram_tensor("f3_own", [cfg.S, D], f32, kind="ExternalInput").ap()
    f2f = nc.dram_tensor("f2_full", [B, D], f32, kind="ExternalInput").ap()
    f3f = nc.dram_tensor("f3_full", [B, D], f32, kind="ExternalInput").ap()
    inp = nc.dram_tensor("inp_own", [cfg.S, D], f32, kind="ExternalInput").ap()
    tgt = nc.dram_tensor("tgt_own", [cfg.S, D], f32, kind="ExternalInput").ap()

    rowparts_d = nc.dram_tensor("rowparts", [P, 3 * MT * NT], f32, kind="ExternalOutput").ap()
    colparts_d = nc.dram_tensor("colparts", [1, 3 * B], f32, kind="ExternalOutput").ap()
    nsq_d = nc.dram_tensor("nsq_own", [P, 3 * MT], f32, kind="ExternalOutput").ap()
    dots_d = nc.dram_tensor("dots_own", [P, 3 * MT], f32, kind="ExternalOutput").ap()
    poi_d = nc.dram_tensor("poi", [P, 2 * MT], f32, kind="ExternalOutput").ap()

    own_dram = [f1o, f2o, f3o]

    with tile.TileContext(nc) as tc:
        with (
            tc.tile_pool(name="const", bufs=1) as const_pool,
            tc.tile_pool(name="persist", bufs=1) as persist,
            tc.tile_pool(name="stage", bufs=6) as stage,
            tc.tile_pool(name="stage16", bufs=4) as stage16,
            tc.tile_pool(name="junk", bufs=2) as junkp,
            tc.tile_pool(name="rhs", bufs=3) as rhsp,
            tc.tile_pool(name="exps", bufs=5) as expp,
            tc.tile_pool(name="small", bufs=6) as smallp,
            tc.tile_pool(name="colpp", bufs=2) as colpp,
            tc.tile_pool(name="dscr", bufs=3, space="DRAM") as dramp,
            tc.tile_pool(name="ps_s", bufs=4, space="PSUM") as ps_s,
            tc.tile_pool(name="ps_t", bufs=2, space="PSUM") as ps_t,
            tc.tile_pool(name="ps_c", bufs=2, space="PSUM") as ps_c,
        ):
            identity = const_pool.tile([P, P], bf16)
            make_identity(nc, identity)
            ones = const_pool.tile([P, 1], bf16)
            nc.vector.memset(ones, 1.0)
            eps_bias = const_pool.tile([P, 1], f32)
            nc.vector.memset(eps_bias, EPS_POISSON)

            # persistent accumulators / stats
            zT1_own = persist.tile([P, K, cfg.S], bf16)
            zT2_own = persist.tile([P, K, cfg.S], bf16)
            rowparts = persist.tile([P, 3 * MT * NT], f32)
            nsq_own = persist.tile([P, 3 * MT], f32)
            dots_own = persist.tile([P, 3 * MT], f32)
            poi = persist.tile([P, 2 * MT], f32)
            scaleA = persist.tile([P, 2 * MT], f32)  # (1/T)/||a|| for f1, f2 own rows

            zT_own = [zT1_own, zT2_own]

            def rsqrt_act(dst, src, n, tag):
                # dst[:, :n] = 1/sqrt(src) = exp(-0.5*ln(src)); Ln and Exp share
                # one activation table set (patched below), so no table thrash.
                l = smallp.tile([P, n], f32, tag=tag)
                nc.scalar.activation(l, src, AF.Ln)
                nc.scalar.activation(dst, l, AF.Exp, scale=-0.5)

            def transpose_rowtile_pe(rb16, zT_dst, t):
                # rb16: [128 rows, D] bf16 row-major -> zT_dst[:, k, t*128:(t+1)*128]
                tps = ps_t.tile([P, K * P], bf16, tag="tps")
                for k in range(K):
                    nc.tensor.transpose(tps[:, ts(k, P)], rb16[:, ts(k, P)], identity)
                nc.any.tensor_copy(
                    out=zT_dst[:, :, ts(t, P)],
                    in_=tps[:].rearrange("p (k c) -> p k c", k=K),
                )

            # ---------------- Phase 0a: own f1/f2 (matmul-critical) ----------------
            own_rf = {}
            for t in range(MT):
                for fi in range(2):
                    rf = stage.tile([P, D], f32, tag="rowf32")
                    nc.sync.dma_start(rf, own_dram[fi][ts(t, P), :])
                    own_rf[(fi, t)] = rf
                    rb = stage16.tile([P, D], bf16, tag="rowbf16")
                    nc.vector.tensor_scalar_mul(rb, rf, 1.0)
                    jt = junkp.tile([P, D], bf16, tag="junk16")
                    nc.vector.scalar_tensor_tensor(
                        out=jt, in0=rb, scalar=1.0, in1=rb,
                        op0=ALU.mult, op1=ALU.mult,
                        accum_out=nsq_own[:, fi * MT + t : fi * MT + t + 1],
                    )
                    transpose_rowtile_pe(rb, zT_own[fi], t)

            # own-row exp scales: (1/T) * rsqrt(nsq) for f1, f2
            recip_own = smallp.tile([P, 2 * MT], f32, tag="recip_own")
            rsqrt_act(recip_own, nsq_own[:, : 2 * MT], 2 * MT, tag="ln_own")
            nc.vector.tensor_scalar_mul(scaleA, recip_own, 1.0 / TEMPERATURE)

            def phase0_tail():
                # f3 norms, raw diagonal dots, poisson partials (independent of
                # the matmul stream; emitted last to fill idle DVE/ACT time)
                for t in range(MT):
                    rfs = []
                    for fi in range(3):
                        rf = stage.tile([P, D], f32, tag="rowf32")
                        nc.sync.dma_start(rf, own_dram[fi][ts(t, P), :])
                        rfs.append(rf)
                    jt = junkp.tile([P, D], bf16, tag="junk16")
                    nc.vector.scalar_tensor_tensor(
                        out=jt, in0=rfs[2], scalar=1.0, in1=rfs[2],
                        op0=ALU.mult, op1=ALU.mult,
                        accum_out=nsq_own[:, 2 * MT + t : 2 * MT + t + 1],
                    )
                    for pi, (ia, ib) in enumerate(((0, 1), (0, 2), (1, 2))):
                        jt = junkp.tile([P, D], bf16, tag="junk16")
                        nc.vector.scalar_tensor_tensor(
                            out=jt, in0=rfs[ia], scalar=1.0, in1=rfs[ib],
                            op0=ALU.mult, op1=ALU.mult,
                            accum_out=dots_own[:, pi * MT + t : pi * MT + t + 1],
                        )
                    it = stage.tile([P, D], f32, tag="rowf32")
                    tt = stage.tile([P, D], f32, tag="rowf32")
                    nc.sync.dma_start(it, inp[ts(t, P), :])
                    nc.sync.dma_start(tt, tgt[ts(t, P), :])
                    lg = stage.tile([P, D], f32, tag="rowf32")
                    nc.scalar.activation(lg, it, AF.Ln, bias=eps_bias[:, :])
                    jt = junkp.tile([P, D], bf16, tag="junk16")
                    nc.vector.scalar_tensor_tensor(
                        out=jt, in0=tt, scalar=1.0, in1=lg,
                        op0=ALU.mult, op1=ALU.mult,
                        accum_out=poi[:, MT + t : MT + t + 1],
                    )
                    jt2 = junkp.tile([P, D], bf16, tag="junk16")
                    nc.vector.tensor_scalar(
                        out=jt2, in0=it, scalar1=1.0, scalar2=0.0, op0=ALU.mult,
                        op1=ALU.add, accum_out=poi[:, t : t + 1],
                    )

            # ---------------- Phase 1: stream full f2, f3 ----------------
            # b=0 -> f2_full (rhs of pair0), b=1 -> f3_full (rhs of pair1, pair2)
            # Two-pass software pipeline per feature with a lag of LAG tiles:
            #   produce(b, nt): load f32 rows, squared norms, rsqrt, normalize
            #                   to bf16, write to DRAM scratch (row-major)
            #   consume(b, nt): one xbar DMA transpose scratch -> zT tiles,
            #                   then the matmul/exp/colsum block.
            # This keeps the sync DMA FIFO free of long produce->consume chains
            # so the transposes prefetch ahead of the PE stream.
            full_dram = [f2f, f3f]
            partners_of = [[(0, 0)], [(1, 0), (2, 1)]]
            scratch_b = [
                dramp.tile([B, D], bf16, tag=f"scratch{b}", name=f"scratch{b}")
                for b in range(2)
            ]

            def produce(b, nt):
                nsq_nt = smallp.tile([P, ST], f32, tag="small")
                recip = smallp.tile([P, ST], f32, tag="recipnt")
                rf_tiles = []
                for t in range(ST):
                    rf = stage.tile([P, D], f32, tag="rowf32")
                    nc.sync.dma_start(rf, full_dram[b][nt * NTC + t * P : nt * NTC + (t + 1) * P, :])
                    rf_tiles.append(rf)
                    slot = nsq_nt[:, t : t + 1]
                    if t % 2 == 0:
                        jt = junkp.tile([P, D], bf16, tag="junk16")
                        nc.vector.scalar_tensor_tensor(
                            out=jt, in0=rf, scalar=1.0, in1=rf,
                            op0=ALU.mult, op1=ALU.mult, accum_out=slot,
                        )
                    else:
                        jt = junkp.tile([P, D], bf16, tag="junk16")
                        nc.scalar.activation(jt, rf, AF.Square, accum_out=slot)
                rsqrt_act(recip, nsq_nt, ST, tag="ln_nt")
                for t in range(ST):
                    zrow = stage16.tile([P, D], bf16, tag="rowbf16n")
                    nc.vector.tensor_scalar_mul(zrow, rf_tiles[t], recip[:, t : t + 1])
                    nc.gpsimd.dma_start(scratch_b[b][nt * NTC + t * P : nt * NTC + (t + 1) * P, :], zrow)

            def consume(b, nt):
                zT_rhs = rhsp.tile([P, K, NTC], bf16, tag="zTr")
                nc.sync.dma_start_transpose(
                    zT_rhs[:, :, :], scratch_b[b][nt * NTC : (nt + 1) * NTC, :]
                )
                for (pair, a) in partners_of[b]:
                    exp_tiles = []
                    for m in range(MT):
                        ps = ps_s.tile([P, NTC], f32, tag="ps_s")
                        for k in range(K):
                            nc.tensor.matmul(
                                ps,
                                zT_own[a][:, k, ts(m, P)],
                                zT_rhs[:, k, :],
                                start=(k == 0),
                                stop=(k == K - 1),
                            )
                        es = expp.tile([P, NTC], bf16, tag="exps")
                        slot = (pair * MT + m) * NT + nt
                        nc.scalar.activation(
                            es, ps, AF.Exp,
                            scale=scaleA[:, a * MT + m : a * MT + m + 1],
                            accum_out=rowparts[:, slot : slot + 1],
                        )
                        exp_tiles.append(es)
                    cps = ps_c.tile([1, NTC], f32, tag="ps_c")
                    for m in range(MT):
                        nc.tensor.matmul(
                            cps, ones, exp_tiles[m],
                            start=(m == 0), stop=(m == MT - 1),
                        )
                    colp = colpp.tile([1, NTC], f32, tag="colp")
                    nc.any.tensor_copy(out=colp, in_=cps)
                    nc.gpsimd.dma_start(
                        colparts_d[:, pair * B + nt * NTC : pair * B + (nt + 1) * NTC],
                        colp,
                    )

            LAG = 2
            for b in range(2):
                for nt in range(NT + LAG):
                    if nt < NT:
                        produce(b, nt)
                    if nt - LAG >= 0:
                        consume(b, nt - LAG)

            phase0_tail()

            # ---------------- outputs ----------------
            nc.gpsimd.dma_start(rowparts_d, rowparts)
            nc.gpsimd.dma_start(nsq_d, nsq_own)
            nc.gpsimd.dma_start(dots_d, dots_own)
            nc.gpsimd.dma_start(poi_d, poi)

    nc.compile()
    return nc


def make_in_maps(cfg: Cfg, inputs, targets, feature1, feature2, feature3):
    f32 = np.float32
    ac = np.ascontiguousarray
    maps = []
    for c in range(cfg.n_cores):
        sl = slice(c * cfg.S, (c + 1) * cfg.S)
        maps.append({
            "f1_own": ac(feature1[sl], dtype=f32),
            "f2_own": ac(feature2[sl], dtype=f32),
            "f3_own": ac(feature3[sl], dtype=f32),
            "f2_full": ac(feature2, dtype=f32),
            "f3_full": ac(feature3, dtype=f32),
            "inp_own": ac(inputs[sl], dtype=f32),
            "tgt_own": ac(targets[sl], dtype=f32),
        })
    return maps


def combine_results(cfg: Cfg, per_core):
    """per_core: list of dicts with rowparts/colparts/nsq_own/dots_own/poi."""
    B, MT, NT, S = cfg.B, cfg.MT, cfg.NT, cfg.S
    nsq = np.zeros((3, B), np.float64)
    dots = np.zeros((3, B), np.float64)
    rowsum = np.zeros((3, B), np.float64)
    colsum = np.zeros((3, B), np.float64)
    poi_in = 0.0
    poi_tl = 0.0
    for c, r in enumerate(per_core):
        rp = np.asarray(r["rowparts"], np.float64)      # [128, 3*MT*NT]
        cp = np.asarray(r["colparts"], np.float64)[0]   # [3*B]
        nq = np.asarray(r["nsq_own"], np.float64)       # [128, 3*MT]
        dt_ = np.asarray(r["dots_own"], np.float64)
        po = np.asarray(r["poi"], np.float64)           # [128, 2*MT]
        for fi in range(3):
            for t in range(MT):
                nsq[fi, c * S + t * P : c * S + (t + 1) * P] = nq[:, fi * MT + t]
        for pi in range(3):
            for m in range(MT):
                rows = slice(c * S + m * P, c * S + (m + 1) * P)
                dots[pi, rows] = dt_[:, pi * MT + m]
                rowsum[pi, rows] = rp[:, (pi * MT + m) * NT : (pi * MT + m + 1) * NT].sum(axis=1)
            colsum[pi] += cp[pi * B : (pi + 1) * B]
        poi_in += po[:, :MT].sum()
        poi_tl += po[:, MT:].sum()

    na = np.sqrt(nsq)  # [3, B]
    pairs = ((0, 1), (0, 2), (1, 2))
    closs = 0.0
    for pi, (ia, ib) in enumerate(pairs):
        simdiag = dots[pi] / (na[ia] * na[ib])
        loss_i = np.mean(np.log(rowsum[pi]) - simdiag / TEMPERATURE)
        loss_j = np.mean(np.log(colsum[pi]) - simdiag / TEMPERATURE)
        closs += 0.5 * (loss_i + loss_j)
    closs /= 3.0
    p_loss = (poi_in - poi_tl) / (cfg.B * cfg.D)
    total = p_loss + closs
    return (
        np.float32(total),
        np.float32(p_loss),
        np.float32(closs),
    )


_CACHE = {}


def _get_compiled(cfg: Cfg):
    key = (cfg.B, cfg.D, cfg.n_cores, cfg.NTC)
    if key not in _CACHE:
        _CACHE[key] = build_bass(cfg)
    return _CACHE[key]


def kernel(inputs, targets, feature1, feature2, feature3):
    from concourse.bass_utils import run_bass_kernel_spmd

    cfg = Cfg(B=inputs.shape[0], D=inputs.shape[1], n_cores=8, ntc=512)
    nc = _get_compiled(cfg)
    in_maps = make_in_maps(cfg, inputs, targets, feature1, feature2, feature3)
    res = run_bass_kernel_spmd(nc, in_maps, core_ids=list(range(cfg.n_cores)))
    return combine_results(cfg, res.results)


if __name__ == "__main__":
    # smoke test on hardware with full shapes
    rng = np.random.default_rng(0)
    B, D = 4096, 1024
    ins = {
        "inputs": rng.random((B, D), np.float32),
        "targets": rng.random((B, D), np.float32),
        "feature1": rng.standard_normal((B, D), np.float32),
        "feature2": rng.standard_normal((B, D), np.float32),
        "feature3": rng.standard_normal((B, D), np.float32),
    }
    out = kernel(**ins)
    print(out)



# revision 14
# speedup vs baseline: 2.2070x; 2.2070x over previous
"""Trainium2 Bass kernel for nn_CombinedLoss (Poisson + 3-way pairwise CLIP loss).

Strategy (8 NeuronCores, SPMD, no collectives), v2 "flipped orientation":
  - Row-shard the batch: core c owns rows [c*512, (c+1)*512) of every tensor.
  - For each pair (a,b) in {(1,2),(1,3),(2,3)} each core computes the FULL
    column strip S_ab^T[n, m] = <b_n_raw, zhat_a_m> for all 4096 n and its own
    512 m, using fp8(e4m3) DoubleRow matmuls (2x PE throughput, fp32 PSUM):
      lhsT (stationary) = raw fp8 b^T tiles, DMA'd directly from a
             host-side transposed+fp8-cast copy of the full feature
             (no on-device normalization / transpose / scratch roundtrip);
      rhs  (moving)     = own rows of a, normalized (x16) to fp8 and
             PE-transposed on-chip.
  - The missing 1/||b_n|| normalization is per-PSUM-PARTITION in this
    orientation, so it folds into the Exp's per-partition scale:
    exp(S_raw[n,m] / (16 * T * ||b_n||)).  ||b_n||^2 for all n comes from
    fp8 Gram-diagonal matmuls on the already-loaded b^T tiles (diag extracted
    with a DVE multiply-by-identity + free-dim accumulate).
  - Column sums over own m (partial, host-combined) come free via the Exp's
    accum_out.  Row sums over all n are ones-vector DoubleRow matmuls
    accumulated across all 32 n-subtiles in one PSUM bank (rows of one
    [128, 512] PSUM tile at partitions 0/32/64 for the 3 pairs).
  - Poisson partials + raw diagonal dots + own norms via DVE fused
    multiply+reduce on bf16 own slices (host-cast; accumulation in f32).
  - Host does only the O(B) final combine: log of 4096-length sums, means.

HBM traffic per core: ~3 MiB own bf16 + 2 MiB inp/tgt bf16 + 8 MiB fp8
full-transposed features = ~13 MiB (vs ~65 MiB for the v1 scratch-roundtrip
kernel).  PE work: ~82us sim matmuls (fp8) + ~10us gram/rowsum overhead.
"""

import math
import sys

import numpy as np

sys.path.insert(0, "/opt/trn_rl_repo")

P = 128
TEMPERATURE = 0.5
EPS_POISSON = 1e-8
OSCALE = 16.0  # own-side normalized rows scaled by this before fp8 cast


class Cfg:
    def __init__(self, B=4096, D=1024, n_cores=8, ntc=1024):
        self.B = B                  # batch
        self.D = D                  # feature dim
        self.n_cores = n_cores
        self.S = B // n_cores       # own rows per core
        self.MT = self.S // P       # own-row tiles
        self.K = D // P             # contraction subtiles
        self.KP = self.K // 2       # fp8 DoubleRow k-pairs
        self.G = B // P             # n-subtiles over the full batch
        self.NTC = min(ntc, B)      # columns per DMA chunk of b^T
        self.NCH = B // self.NTC    # chunks per feature
        self.SC = self.NTC // P     # n-subtiles per chunk
        assert B % n_cores == 0 and self.S % P == 0 and D % (2 * P) == 0
        assert B % self.NTC == 0 and self.NTC % P == 0 and self.SC % 2 == 0


def _patch_act_tables():
    """Make Bacc's act-table pass pick `natural_log_exp_and_others` for both
    Exp and Ln (they otherwise land in two different sets, and alternating
    Ln/Exp calls reload the 2.7us activation tables every tile)."""
    import functools

    import concourse.hw_specs as hw_specs

    if getattr(hw_specs, "_act_tables_patched", False):
        return
    orig = hw_specs.get_activation_tables

    @functools.cache
    def patched(module_arch):
        tabs = dict(orig(module_arch))
        names = list(tabs.keys())
        if "natural_log_exp_and_others" in tabs:
            combined = tabs["natural_log_exp_and_others"]
            for name in names:
                if name == "natural_log_exp_and_others":
                    break
                if tabs[name] & combined:
                    tabs[name] = tabs[name] - combined
        return tabs

    hw_specs.get_activation_tables = patched
    import concourse.bacc as bacc_mod

    if hasattr(bacc_mod, "get_activation_tables"):
        bacc_mod.get_activation_tables = patched
    hw_specs._act_tables_patched = True


def build_bass(cfg: Cfg):
    """Build the single-core Bass program (same program for all SPMD cores)."""
    import concourse.bacc as bacc
    import concourse.bass as bass
    import concourse.mybir as mybir
    import concourse.tile as tile
    from concourse.masks import make_identity

    _patch_act_tables()

    f32 = mybir.dt.float32
    bf16 = mybir.dt.bfloat16
    fp8 = mybir.dt.float8e4
    AF = mybir.ActivationFunctionType
    ALU = mybir.AluOpType
    DR = mybir.MatmulPerfMode.DoubleRow
    ts = bass.ts

    B, D, S, MT, K, KP, G, NTC, NCH, SC = (
        cfg.B, cfg.D, cfg.S, cfg.MT, cfg.K, cfg.KP, cfg.G, cfg.NTC, cfg.NCH, cfg.SC,
    )

    nc = bacc.Bacc(
        "TRN2",
        target_bir_lowering=False,
        debug=False,
        enable_asserts=False,
        num_devices=cfg.n_cores,
    )

    # ---- IO ----
    f1o = nc.dram_tensor("f1_own", [S, D], bf16, kind="ExternalInput").ap()
    f2o = nc.dram_tensor("f2_own", [S, D], bf16, kind="ExternalInput").ap()
    f3o = nc.dram_tensor("f3_own", [S, D], bf16, kind="ExternalInput").ap()
    inp = nc.dram_tensor("inp_own", [S, D], bf16, kind="ExternalInput").ap()
    tgt = nc.dram_tensor("tgt_own", [S, D], bf16, kind="ExternalInput").ap()
    f2T = nc.dram_tensor("f2T8", [D, B], fp8, kind="ExternalInput").ap()
    f3T = nc.dram_tensor("f3T8", [D, B], fp8, kind="ExternalInput").ap()

    nsq_d = nc.dram_tensor("nsq_own", [P, 3 * MT], f32, kind="ExternalOutput").ap()
    dots_d = nc.dram_tensor("dots_own", [P, 3 * MT], f32, kind="ExternalOutput").ap()
    poi_d = nc.dram_tensor("poi", [P, 2 * MT], f32, kind="ExternalOutput").ap()
    rows_d = nc.dram_tensor("rowsums", [1, 3 * S], f32, kind="ExternalOutput").ap()
    colp_d = nc.dram_tensor("colparts", [P, 3 * G], f32, kind="ExternalOutput").ap()

    own_dram = [f1o, f2o, f3o]
    fT_dram = [f2T, f3T]
    # pairs as (pair_index, own_feature a) grouped by the full-side feature b
    pairs_of_b = [[(0, 0)], [(1, 0), (2, 1)]]  # b=f2: (f1,f2); b=f3: (f1,f3),(f2,f3)

    with tile.TileContext(nc) as tc:
        with (
            tc.tile_pool(name="const", bufs=1) as const_pool,
            tc.tile_pool(name="persist", bufs=1) as persist,
            tc.tile_pool(name="own", bufs=1) as ownp,
            tc.tile_pool(name="stage16", bufs=4) as stage16,
            tc.tile_pool(name="lg", bufs=2) as lgp,
            tc.tile_pool(name="junk", bufs=2) as junkp,
            tc.tile_pool(name="exps", bufs=3) as expp,
            tc.tile_pool(name="small", bufs=6) as smallp,
            tc.tile_pool(name="ps_s", bufs=3, space="PSUM") as ps_s,
            tc.tile_pool(name="ps_row", bufs=1, space="PSUM") as ps_rowp,
            tc.tile_pool(name="ps_g", bufs=2, space="PSUM") as ps_g,
            tc.tile_pool(name="ps_t", bufs=2, space="PSUM") as ps_t,
        ):
            ident16 = const_pool.tile([P, P], bf16)
            make_identity(nc, ident16)
            ident32 = const_pool.tile([P, P], f32)
            make_identity(nc, ident32)
            # two ones per partition, 16B apart (DoubleRow weight APs need the
            # k-pair stride 16B-aligned)
            ones8_pad = const_pool.tile([P, 2, 16], fp8)
            nc.vector.memset(ones8_pad, 1.0)
            ones8 = ones8_pad[:, :, 0:1]
            eps_bias = const_pool.tile([P, 1], f32)
            nc.vector.memset(eps_bias, EPS_POISSON)
            ln16_bias = const_pool.tile([P, 1], f32)
            nc.vector.memset(ln16_bias, math.log(OSCALE))
            lnbt_bias = const_pool.tile([P, 1], f32)
            nc.vector.memset(lnbt_bias, -math.log(OSCALE * TEMPERATURE))

            # persistent state
            fT_sb = [persist.tile([P, K, B], fp8, name=f"fT_sb{b}") for b in range(2)]
            zT_own = [persist.tile([P, K, S], fp8, name=f"zT_own{a}") for a in range(2)]
            nsq_own = persist.tile([P, 3 * MT], f32)
            dots_own = persist.tile([P, 3 * MT], f32)
            poi = persist.tile([P, 2 * MT], f32)
            scale16 = persist.tile([P, 2 * MT], f32)   # 16/||a|| for f1,f2 own
            bnsq = persist.tile([P, 2 * G], f32)       # ||b_n||^2 (fp8 data)
            bscale = persist.tile([P, 2 * G], f32)     # 1/(16*T*||b_n||)
            colp_sb = persist.tile([P, 3 * G], f32)

            # ---- prefetch the full transposed fp8 features (scalar HWDGE q) ----
            for b in range(2):
                src = fT_dram[b].rearrange("(k p) n -> p k n", p=P)
                for ch in range(NCH):
                    nc.scalar.dma_start(
                        fT_sb[b][:, :, ts(ch, NTC)], src[:, :, ts(ch, NTC)]
                    )

            # ---- phase A: own rows (sync queue), norms, normalize+transpose ----
            rf_own = {}
            for t in range(MT):
                for fi in range(3):
                    rf = ownp.tile([P, D], bf16, tag=f"rf{fi}_{t}")
                    nc.sync.dma_start(rf, own_dram[fi][ts(t, P), :])
                    rf_own[(fi, t)] = rf
                    jt = junkp.tile([P, D], bf16, tag="junk16")
                    nc.vector.scalar_tensor_tensor(
                        out=jt, in0=rf, scalar=1.0, in1=rf,
                        op0=ALU.mult, op1=ALU.mult,
                        accum_out=nsq_own[:, fi * MT + t : fi * MT + t + 1],
                    )

            # scale16 = 16 / ||a||  (ACT: exp(-0.5*ln(nsq) + ln 16))
            lnq = smallp.tile([P, 2 * MT], f32, tag="ln_own")
            nc.scalar.activation(lnq, nsq_own[:, : 2 * MT], AF.Ln)
            nc.scalar.activation(
                scale16, lnq, AF.Exp, scale=-0.5, bias=ln16_bias[:, :]
            )

            # normalize own f1/f2 rows (x16, bf16) then PE-transpose; the
            # PSUM->SBUF copy casts to fp8.
            for fi in range(2):
                for t in range(MT):
                    zrow = stage16.tile([P, D], bf16, tag="zhat16")
                    nc.vector.tensor_scalar_mul(
                        zrow, rf_own[(fi, t)], scale16[:, fi * MT + t : fi * MT + t + 1]
                    )
                    tps = ps_t.tile([P, K * P], bf16, tag="tps")
                    for k in range(K):
                        nc.tensor.transpose(tps[:, ts(k, P)], zrow[:, ts(k, P)], ident16)
                    nc.vector.tensor_copy(
                        out=zT_own[fi][:, :, ts(t, P)],
                        in_=tps[:].rearrange("p (k c) -> p k c", k=K),
                    )

            # ---- phase B: stream both b features ----
            rows_acc = persist.tile([1, 3 * S], f32)
            nc.vector.memset(rows_acc, 0.0)

            for b in range(2):
                for ch in range(NCH):
                    # b-norms for this chunk: fp8 Gram diagonals
                    for s in range(SC):
                        g = ch * SC + s
                        gram = ps_g.tile([P, P], f32, tag="gram")
                        bsub = fT_sb[b][:, :, ts(g, P)]
                        for j in range(KP):
                            nc.tensor.matmul(
                                gram,
                                bsub[:, 2 * j : 2 * j + 2, :],
                                bsub[:, 2 * j : 2 * j + 2, :],
                                start=(j == 0), stop=(j == KP - 1),
                                perf_mode=DR,
                            )
                        j8 = junkp.tile([P, P], bf16, tag="junkg")
                        nc.vector.scalar_tensor_tensor(
                            out=j8, in0=gram, scalar=1.0, in1=ident32,
                            op0=ALU.mult, op1=ALU.mult,
                            accum_out=bnsq[:, b * G + g : b * G + g + 1],
                        )
                    lnb = smallp.tile([P, SC], f32, tag="lnb")
                    sl = slice(b * G + ch * SC, b * G + (ch + 1) * SC)
                    nc.scalar.activation(lnb, bnsq[:, sl], AF.Ln)
                    nc.scalar.activation(
                        bscale[:, sl], lnb, AF.Exp,
                        scale=-0.5, bias=lnbt_bias[:, :],
                    )

                    # sim matmuls + exp + row/col sums
                    for (pair, a) in pairs_of_b[b]:
                        e2 = None
                        rp = ps_rowp.tile([1, S], f32, tag="rp")
                        for s in range(SC):
                            g = ch * SC + s
                            ps = ps_s.tile([P, S], f32, tag="ps_s")
                            bsub = fT_sb[b][:, :, ts(g, P)]
                            for j in range(KP):
                                nc.tensor.matmul(
                                    ps,
                                    bsub[:, 2 * j : 2 * j + 2, :],
                                    zT_own[a][:, 2 * j : 2 * j + 2, :],
                                    start=(j == 0), stop=(j == KP - 1),
                                    perf_mode=DR,
                                )
                            if s % 2 == 0:
                                e2 = expp.tile([P, 2, S], fp8, tag="exp8")
                            nc.scalar.activation(
                                e2[:, s % 2, :], ps, AF.Exp,
                                scale=bscale[:, b * G + g : b * G + g + 1],
                                accum_out=colp_sb[:, pair * G + g : pair * G + g + 1],
                            )
                            if s % 2 == 1:
                                nc.tensor.matmul(
                                    rp,
                                    ones8,
                                    e2[:, :, :],
                                    start=(s == 1),
                                    stop=(s == SC - 1),
                                    perf_mode=DR,
                                    skip_group_check=True,
                                )
                        acc = rows_acc[:, pair * S : (pair + 1) * S]
                        nc.vector.tensor_tensor(
                            out=acc, in0=rp, in1=acc, op=ALU.add
                        )

            # ---- tail: diag dots + poisson (fills idle DVE/ACT time) ----
            for t in range(MT):
                for pi, (ia, ib) in enumerate(((0, 1), (0, 2), (1, 2))):
                    jt = junkp.tile([P, D], bf16, tag="junk16")
                    nc.vector.scalar_tensor_tensor(
                        out=jt, in0=rf_own[(ia, t)], scalar=1.0, in1=rf_own[(ib, t)],
                        op0=ALU.mult, op1=ALU.mult,
                        accum_out=dots_own[:, pi * MT + t : pi * MT + t + 1],
                    )
                it = stage16.tile([P, D], bf16, tag="inpt")
                tt = stage16.tile([P, D], bf16, tag="tgtt")
                nc.sync.dma_start(it, inp[ts(t, P), :])
                nc.sync.dma_start(tt, tgt[ts(t, P), :])
                lg = lgp.tile([P, D], f32, tag="lg")
                nc.scalar.activation(lg, it, AF.Ln, bias=eps_bias[:, :])
                jt = junkp.tile([P, D], bf16, tag="junk16")
                nc.vector.scalar_tensor_tensor(
                    out=jt, in0=tt, scalar=1.0, in1=lg,
                    op0=ALU.mult, op1=ALU.mult,
                    accum_out=poi[:, MT + t : MT + t + 1],
                )
                jt2 = junkp.tile([P, D], bf16, tag="junk16")
                nc.vector.tensor_scalar(
                    out=jt2, in0=it, scalar1=1.0, scalar2=0.0, op0=ALU.mult,
                    op1=ALU.add, accum_out=poi[:, t : t + 1],
                )

            # ---- outputs ----
            nc.gpsimd.dma_start(rows_d, rows_acc)
            nc.gpsimd.dma_start(colp_d, colp_sb)
            nc.gpsimd.dma_start(nsq_d, nsq_own)
            nc.gpsimd.dma_start(dots_d, dots_own)
            nc.gpsimd.dma_start(poi_d, poi)

    nc.compile()
    return nc


def make_in_maps(cfg: Cfg, inputs, targets, feature1, feature2, feature3):
    import ml_dtypes

    bf16 = ml_dtypes.bfloat16
    fp8 = ml_dtypes.float8_e4m3
    ac = np.ascontiguousarray
    # full transposed fp8 copies (shared across cores)
    f2T8 = ac(np.asarray(feature2, dtype=np.float32).T).astype(fp8)
    f3T8 = ac(np.asarray(feature3, dtype=np.float32).T).astype(fp8)
    maps = []
    for c in range(cfg.n_cores):
        sl = slice(c * cfg.S, (c + 1) * cfg.S)
        maps.append({
            "f1_own": ac(feature1[sl]).astype(bf16),
            "f2_own": ac(feature2[sl]).astype(bf16),
            "f3_own": ac(feature3[sl]).astype(bf16),
            "inp_own": ac(inputs[sl]).astype(bf16),
            "tgt_own": ac(targets[sl]).astype(bf16),
            "f2T8": f2T8,
            "f3T8": f3T8,
        })
    return maps


def combine_results(cfg: Cfg, per_core):
    """per_core: list of dicts with rowsums/colparts/nsq_own/dots_own/poi."""
    B, MT, S, G = cfg.B, cfg.MT, cfg.S, cfg.G
    nsq = np.zeros((3, B), np.float64)
    dots = np.zeros((3, B), np.float64)
    rowsum = np.zeros((3, B), np.float64)
    colsum = np.zeros((3, B), np.float64)
    poi_in = 0.0
    poi_tl = 0.0
    for c, r in enumerate(per_core):
        rs = np.asarray(r["rowsums"], np.float64).reshape(3, S)
        cp = np.asarray(r["colparts"], np.float64)      # [128, 3*G]
        nq = np.asarray(r["nsq_own"], np.float64)       # [128, 3*MT]
        dt_ = np.asarray(r["dots_own"], np.float64)
        po = np.asarray(r["poi"], np.float64)           # [128, 2*MT]
        for fi in range(3):
            for t in range(MT):
                rows = slice(c * S + t * P, c * S + (t + 1) * P)
                nsq[fi, rows] = nq[:, fi * MT + t]
        for pi in range(3):
            rowsum[pi, c * S : (c + 1) * S] = rs[pi]
            for t in range(MT):
                rows = slice(c * S + t * P, c * S + (t + 1) * P)
                dots[pi, rows] = dt_[:, pi * MT + t]
            # colparts: n = g*128 + lane
            colsum[pi] += cp[:, pi * G : (pi + 1) * G].T.reshape(-1)
        poi_in += po[:, :MT].sum()
        poi_tl += po[:, MT:].sum()

    na = np.sqrt(nsq)  # [3, B]
    pairs = ((0, 1), (0, 2), (1, 2))
    closs = 0.0
    for pi, (ia, ib) in enumerate(pairs):
        simdiag = dots[pi] / (na[ia] * na[ib])
        loss_i = np.mean(np.log(rowsum[pi]) - simdiag / TEMPERATURE)
        loss_j = np.mean(np.log(colsum[pi]) - simdiag / TEMPERATURE)
        closs += 0.5 * (loss_i + loss_j)
    closs /= 3.0
    p_loss = (poi_in - poi_tl) / (cfg.B * cfg.D)
    total = p_loss + closs
    return (
        np.float32(total),
        np.float32(p_loss),
        np.float32(closs),
    )


_CACHE = {}


def _get_compiled(cfg: Cfg):
    key = (cfg.B, cfg.D, cfg.n_cores, cfg.NTC)
    if key not in _CACHE:
        _CACHE[key] = build_bass(cfg)
    return _CACHE[key]


def kernel(inputs, targets, feature1, feature2, feature3):
    from concourse.bass_utils import run_bass_kernel_spmd

    cfg = Cfg(B=inputs.shape[0], D=inputs.shape[1], n_cores=8)
    nc = _get_compiled(cfg)
    in_maps = make_in_maps(cfg, inputs, targets, feature1, feature2, feature3)
    res = run_bass_kernel_spmd(nc, in_maps, core_ids=list(range(cfg.n_cores)))
    return combine_results(cfg, res.results)


if __name__ == "__main__":
    # smoke test on hardware with full shapes
    rng = np.random.default_rng(0)
    B, D = 4096, 1024
    ins = {
        "inputs": rng.random((B, D), np.float32),
        "targets": rng.random((B, D), np.float32),
        "feature1": rng.standard_normal((B, D), np.float32),
        "feature2": rng.standard_normal((B, D), np.float32),
        "feature3": rng.standard_normal((B, D), np.float32),
    }
    out = kernel(**ins)
    print(out)
